# revision 3
# baseline (speedup 1.0000x reference)
"""Linformer attention TRN2 Bass kernel.

Problem: nn_LinformerAttention (B=4, L=4096, D=1024, NH=16, DH=64, k=128).

Sharding: 8 cores = batch(4) x head-group(2). Core c handles batch c%4 and
heads (c//4)*8 .. +8, producing out[b, :, hg*512:(hg+1)*512]. Slices are
disjoint -> no collectives; host reassembles.

All matmul operands are bf16 (fp32 PSUM accumulation). On TRN2, fp32r
matmuls run in fp32_mode=HIGH at ~2 cycles/row and fp32 matmuls run
two-pass LOW_HIGH at 4+ cycles/row; bf16 streams 1 row/cycle. rel-err
budget is 2e-2 and bf16 lands ~6e-3, so this halves PE time for free.
fp8 was measured (host-sim) at 5.8e-2 even with power-of-2 pre-scaling
to dodge denormals — over budget, rejected.

Device algorithm per core, two passes over 8 l-chunks of 512:
  pass A (per chunk): K = x @ Wk.T + bk and V likewise (PSUM accum over 8
    d-subtiles, all K matmuls before all V matmuls so startup only gates
    on Wk; bias+cast to bf16 on DVE); KVp[h] += E_h-chunk.T @ [K|V]
    (Linformer projection accumulated in SBUF, stored per head-PAIR so one
    PE transpose per pair puts odd heads' Kp.T rows at partitions 64..127).
  between passes: KpT per head into zero-padded [128, kk] tiles (head h at
    partitions (h%2)*64..+64, rest ZERO so the dot matmul can contract the
    full 128-partition Q tile); Vp_aug = [Vp | ones].
  pass B (per chunk; x re-DMA'd — cheaper than keeping Q resident):
    - Q.T-chunk = Wq @ x.T + bq (scale 1/sqrt(dh) folded into Wq/bq on
      host), straight into SBUF, consumed immediately
    - per head (software-pipelined so dot-matmuls stream on PE while exp
      runs on Scalar): dotT = KpT.T @ Q.T-chunk; expT = exp(dotT) (ACT,
      logits are small by construction, exp is safe); Xo_aug[l, lt, 65] =
      expT-tile.T @ Vp_aug into ONE PSUM bank (col 64 = softmax denom);
      one batched reciprocal [128,4] + one broadcast-mult [128,4,64]
    - out DMA per head-pair (quarter chunk) so DMA overlaps compute; the
      final chunk uses per-head DMAs alternating sync/gpsimd queues to
      shorten the drain tail.

Scheduling notes (from perfetto traces): DMA descriptor generation
(DIRECT2D) costs ~0.7us per instruction on the issuing engine queue and
was the startup gate. Weights are host-pre-permuted to [128, dc, j] so
each load is one descriptor per partition, and the initial loads are
spread across queues: Wk/biases on Sync, Wv on Vector, E-chunk0 + Wq on
Scalar, x on GpSimd. psQ draws from the psB pool so psD has 4 dedicated
PSUM banks (dot_h only waits on exp_{h-4}).

Host prep (numpy, outside HW-timed region): x[b].T pre-tiled per (chunk,
l-tile) for parallel-issue DMAs, W slices pre-transposed+pre-permuted
(+1/8 scale on Wq), E head-slices pre-transposed, all cast to bf16 (bq
tile stays fp32).
"""

import sys

sys.path.insert(0, "/opt/trn_rl_repo")

import math
from contextlib import ExitStack

import numpy as np
import ml_dtypes

import json

import concourse.bass as bass
import concourse.bass2jax as bass2jax
import concourse.mybir as mybir
import concourse.tile as tile
from concourse.bass_utils import compile_bir_kernel as _orig_compile_bir_kernel
from concourse.bass_utils import run_bass_kernel_spmd
from concourse.masks import make_identity


def _split_multiwaits(bir_json_bytes):
    """This container's walrus encodes at most ONE sync wait per engine
    instruction ("Too many sync wait commands" otherwise), while Tile emits
    multi-wait instructions. Hoist extra waits onto single-wait
    EventSemaphore carrier instructions placed just before, on the same
    engine queue — semantically identical stalling."""
    bj = json.loads(bir_json_bytes)
    for fn in bj["functions"]:
        for blk in fn["blocks"]:
            out = []
            for inst in blk["instructions"]:
                si = inst.get("sync_info")
                waits = (si or {}).get("on_wait") or []
                if si and len(waits) > 1:
                    for wi, w in enumerate(waits[:-1]):
                        out.append(
                            {
                                "debug": inst.get("debug", 0),
                                "engine": inst.get("engine"),
                                "ins": [],
                                "outs": [],
                                "name": inst["name"] + "-w%d" % wi,
                                "opcode": "EventSemaphore",
                                "sync_info": {"on_update": [], "on_wait": [w]},
                            }
                        )
                    si["on_wait"] = [waits[-1]]
                out.append(inst)
            blk["instructions"] = out
    return json.dumps(bj).encode()


def _patched_compile_bir_kernel(bir_json, tmpdir, neff_name="file.neff"):
    return _orig_compile_bir_kernel(_split_multiwaits(bir_json), tmpdir, neff_name)


bass2jax.compile_bir_kernel = _patched_compile_bir_kernel

B, L, D = 4, 4096, 1024
NH, DH, KK = 16, 64, 128
NCORES = 8
HGS = 2  # head groups
H = NH // HGS  # 8 local heads per core
J = H * DH  # 512 output columns per core
P = 128
LCH = 512  # l-chunk
NLC = L // LCH  # 8
DC = D // P  # 8 contraction subtiles
JT = J // P  # 4
LT4 = LCH // P  # 4 l-tiles per chunk
F32 = mybir.dt.float32
BF16 = mybir.dt.bfloat16

TRACE = False  # test.py sets True to collect a profile
LAST_RESULTS = None  # BassKernelResults of the last kernel() call

_PROGRAM = None


def _build_program():
    nc = bass.Bass()
    # x pre-tiled on host: [lc, lt, pi, dc, ll] so each (lc, lt) piece is one
    # DMA with 2 KiB/partition contiguous lines, and pieces spread across
    # DMA queues (the single-queue 1 MiB chunk DMA was gating startup).
    xT = nc.declare_dram_parameter("xT", [NLC, LCH // P, P, D // P, P], BF16, isOutput=False)
    # weights pre-permuted on host to [pi, dc, j]: contiguous per-partition
    # lines -> one descriptor per partition per DMA (cheap DIRECT2D gen).
    wqP = nc.declare_dram_parameter("wqP", [P, DC, J], BF16, isOutput=False)
    wkP = nc.declare_dram_parameter("wkP", [P, DC, J], BF16, isOutput=False)
    wvP = nc.declare_dram_parameter("wvP", [P, DC, J], BF16, isOutput=False)
    bqT = nc.declare_dram_parameter("bqT", [P, JT], F32, isOutput=False)
    bkB = nc.declare_dram_parameter("bkB", [P, J], BF16, isOutput=False)
    bvB = nc.declare_dram_parameter("bvB", [P, J], BF16, isOutput=False)
    eT = nc.declare_dram_parameter("eT", [NLC, P, H, LT4, KK], BF16, isOutput=False)
    out = nc.declare_dram_parameter("out", [L, J], F32, isOutput=True)

    add = mybir.AluOpType.add
    mult = mybir.AluOpType.mult

    with tile.TileContext(nc) as tc:
        with ExitStack() as ctx:
            const = ctx.enter_context(tc.tile_pool(name="const", bufs=1))
            xpool = ctx.enter_context(tc.tile_pool(name="x", bufs=2))
            kvpool = ctx.enter_context(tc.tile_pool(name="kv", bufs=8))
            epool = ctx.enter_context(tc.tile_pool(name="e", bufs=2))
            qtpool = ctx.enter_context(tc.tile_pool(name="qt", bufs=2))
            exppool = ctx.enter_context(tc.tile_pool(name="ex", bufs=4))
            outpool = ctx.enter_context(tc.tile_pool(name="ot", bufs=2))
            recpool = ctx.enter_context(tc.tile_pool(name="rc", bufs=4))
            psA = ctx.enter_context(tc.tile_pool(name="psA", bufs=4, space="PSUM"))
            psB = ctx.enter_context(tc.tile_pool(name="psB", bufs=2, space="PSUM"))
            psXp = ctx.enter_context(tc.tile_pool(name="psX", bufs=2, space="PSUM"))

            # ---- constants resident in SBUF
            wq_sb = const.tile([P, DC, J], BF16, tag="wq")
            wk_sb = const.tile([P, DC, J], BF16, tag="wk")
            wv_sb = const.tile([P, DC, J], BF16, tag="wv")
            # dc=0 slices land first so the first projection matmuls start
            # a fraction of a weight-load into the kernel instead of waiting
            # for the full 1 MiB per weight. Wk on the Sync queue, Wv on the
            # Scalar queue so descriptor generation runs in parallel (DMAs
            # can only issue from the Sync/Scalar/GpSimd queues).
            nc.sync.dma_start(wk_sb[:, 0:1, :], wkP[:, 0:1, :])
            nc.sync.dma_start(wk_sb[:, 1:DC, :], wkP[:, 1:DC, :])
            nc.scalar.dma_start(wv_sb[:, 0:1, :], wvP[:, 0:1, :])
            nc.scalar.dma_start(wv_sb[:, 1:DC, :], wvP[:, 1:DC, :])
            bqT_sb = const.tile([P, JT], F32, tag="bqT")
            bkB_sb = const.tile([P, J], BF16, tag="bkB")
            bvB_sb = const.tile([P, J], BF16, tag="bvB")
            nc.sync.dma_start(bkB_sb[:], bkB[:, :])
            nc.sync.dma_start(bvB_sb[:], bvB[:, :])
            ident = const.tile([P, P], F32, tag="ident")
            make_identity(nc, ident[:])

            # K/V Linformer accumulators, one per head PAIR: [kk, {K,V}, dh-pair]
            kvpP = [const.tile([P, 2, P], F32, tag=f"kvp{t}", name=f"kvp{t}") for t in range(JT)]
            # per-head Kp.T for the dot matmul: head h occupies partitions
            # (h%2)*64..+64, the other 64 partitions are ZERO so the matmul can
            # contract all 128 partitions of the shared Q tile.
            kpT = [const.tile([P, KK], BF16, tag=f"kpT{h}", name=f"kpT{h}") for h in range(H)]
            vpa = [const.tile([P, DH + 1], BF16, tag=f"vpa{h}", name=f"vpa{h}") for h in range(H)]
            for h in range(H):
                b0z = ((h + 1) % 2) * DH  # the half that must stay zero
                nc.vector.memset(kpT[h][b0z : b0z + DH, :], 0.0)

            outr = out[:, :].rearrange("(lo li) j -> li lo j", li=P)

            # ---- pass A: K/V projections + Linformer reduction
            for lc in range(NLC):
                x_sb = xpool.tile([P, LT4, DC, P], BF16, tag="x")
                if lc == 0:
                    # dc=0 sliver of the first l-tile first: the very first
                    # matmul then gates on 32 KiB, not 256 KiB.
                    nc.gpsimd.dma_start(x_sb[:, 0, 0:1, :], xT[0, 0, :, 0:1, :])
                    nc.gpsimd.dma_start(x_sb[:, 0, 1:DC, :], xT[0, 0, :, 1:DC, :])
                    for lt in range(1, LT4):
                        nc.gpsimd.dma_start(x_sb[:, lt, :, :], xT[lc, lt])
                    # E for chunk 0 on the otherwise-idle Scalar queue so it
                    # overlaps the weight loads on Sync/Vector.
                    e_sb0 = epool.tile([P, H, LT4, KK], BF16, tag="e")
                    nc.scalar.dma_start(e_sb0[:, 0 : H // 2], eT[0, :, 0 : H // 2])
                    nc.scalar.dma_start(e_sb0[:, H // 2 : H], eT[0, :, H // 2 : H])
                else:
                    for lt in range(LT4):
                        nc.gpsimd.dma_start(x_sb[:, lt, :, :], xT[lc, lt])
                kv_tiles = []
                for lt in range(LT4):
                    psK = psA.tile([P, LCH], F32, tag="big")
                    psV = psA.tile([P, LCH], F32, tag="big")
                    # all K matmuls first: V waits on the Wv load at startup
                    for dc in range(DC):
                        nc.tensor.matmul(
                            psK[:], x_sb[:, lt, dc, :],
                            wk_sb[:, dc, :],
                            start=(dc == 0), stop=(dc == DC - 1),
                        )
                    for dc in range(DC):
                        nc.tensor.matmul(
                            psV[:], x_sb[:, lt, dc, :],
                            wv_sb[:, dc, :],
                            start=(dc == 0), stop=(dc == DC - 1),
                        )
                    kv_sb = kvpool.tile([P, 2, LCH], BF16, tag="kv")
                    nc.any.tensor_tensor(kv_sb[:, 0, :], psK[:], bkB_sb[:], add)
                    nc.any.tensor_tensor(kv_sb[:, 1, :], psV[:], bvB_sb[:], add)
                    kv_tiles.append(kv_sb)
                if lc == 0:
                    e_sb = e_sb0
                else:
                    e_sb = epool.tile([P, H, LT4, KK], BF16, tag="e")
                    nc.sync.dma_start(e_sb[:, 0 : H // 2], eT[lc, :, 0 : H // 2])
                    nc.sync.dma_start(e_sb[:, H // 2 : H], eT[lc, :, H // 2 : H])
                for h in range(H):
                    par = h % 2
                    acc = kvpP[h // 2][:, :, par * DH : (par + 1) * DH]
                    psKV = psB.tile([P, 2, DH], F32, tag="big")
                    for lt in range(LT4):
                        nc.tensor.matmul(
                            psKV[:], e_sb[:, h, lt, :],
                            kv_tiles[lt][:, :, h * DH : (h + 1) * DH],
                            start=(lt == 0), stop=(lt == LT4 - 1),
                        )
                    if lc == 0:
                        nc.any.tensor_copy(acc, psKV[:])
                    else:
                        nc.any.tensor_tensor(acc, acc, psKV[:], add)
                if lc == 0:
                    # wq / bqT are first needed in pass B (~t+120us); issue on
                    # the Scalar queue (idle in pass A) after chunk 0's E.
                    nc.scalar.dma_start(wq_sb[:], wqP[:, :, :])
                    nc.scalar.dma_start(bqT_sb[:], bqT[:, :])

            # ---- between passes: Kp.T / Vp_aug staging
            for t in range(JT):
                # transpose both heads of the pair at once: [kk, dh2] -> [dh2, kk];
                # odd head's rows land at partitions 64..127 by construction
                psT = psB.tile([P, KK], F32, tag="big")
                nc.tensor.transpose(psT[:], kvpP[t][:, 0, :], ident[:])
                for par in range(2):
                    h = 2 * t + par
                    b0 = par * DH
                    nc.any.tensor_copy(kpT[h][b0 : b0 + DH, :], psT[b0 : b0 + DH, :])
                    nc.any.tensor_copy(
                        vpa[h][:, 0:DH], kvpP[t][:, 1, b0 : b0 + DH]
                    )
                    nc.vector.memset(vpa[h][:, DH : DH + 1], 1.0)

            # ---- pass B: Q projection fused with attention, per chunk
            DEPTH = 3  # psD/exp issued this many heads ahead of psX
            for lc in range(NLC):
                x_sb = xpool.tile([P, LT4, DC, P], BF16, tag="x")
                for lt in range(LT4):
                    nc.gpsimd.dma_start(x_sb[:, lt, :, :], xT[lc, lt])
                qt = qtpool.tile([P, JT, LCH], BF16, tag="qt")
                for jt in range(JT):
                    # psQ draws from psB so psD keeps all 4 psA banks: the
                    # dot matmul for head h then only waits on exp_{h-4}.
                    psQ = psB.tile([P, LCH], F32, tag="big")
                    for dc in range(DC):
                        nc.tensor.matmul(
                            psQ[:], wq_sb[:, dc, jt * P : (jt + 1) * P],
                            x_sb[:, :, dc, :],
                            start=(dc == 0), stop=(dc == DC - 1),
                        )
                    nc.vector.tensor_scalar(
                        qt[:, jt, :], psQ[:], bqT_sb[:, jt : jt + 1], None, add
                    )
                ot = outpool.tile([P, LT4, J], F32, tag="ot")
                exs = [None] * H
                for hh in range(H + DEPTH):
                    if hh < H:
                        h = hh
                        psD = psA.tile([P, LCH], F32, tag="big")
                        nc.tensor.matmul(
                            psD[:], kpT[h][:],
                            qt[:, h // 2, :],
                            start=True, stop=True,
                        )
                        ex = exppool.tile([P, LCH], BF16, tag="ex")
                        nc.scalar.activation(
                            ex[:], psD[:], mybir.ActivationFunctionType.Exp
                        )
                        exs[h] = ex
                    if hh >= DEPTH:
                        h = hh - DEPTH
                        ex = exs[h]
                        psX = psXp.tile([P, LT4, DH + 1], F32, tag="x4")
                        for lt in range(LT4):
                            nc.tensor.matmul(
                                psX[:, lt, :], ex[:, lt * P : (lt + 1) * P],
                                vpa[h][:],
                                start=True, stop=True,
                            )
                        rc = recpool.tile([P, LT4, 1], F32, tag="rc")
                        nc.vector.reciprocal(rc[:], psX[:, :, DH : DH + 1])
                        nc.vector.tensor_tensor(
                            ot[:, :, h * DH : (h + 1) * DH],
                            psX[:, :, 0:DH],
                            rc[:].to_broadcast([P, LT4, DH]),
                            mult,
                        )
                        if lc < NLC - 1:
                            if h % 2 == 1:
                                j0 = (h - 1) * DH
                                nc.sync.dma_start(
                                    outr[:, lc * LT4 : (lc + 1) * LT4, j0 : j0 + 2 * DH],
                                    ot[:, :, j0 : j0 + 2 * DH],
                                )
                        else:
                            # final chunk: per-head DMAs on alternating queues
                            # so the drain tail is as short as possible
                            j0 = h * DH
                            eng = nc.gpsimd if h % 2 == 0 else nc.sync
                            eng.dma_start(
                                outr[:, lc * LT4 : (lc + 1) * LT4, j0 : j0 + DH],
                                ot[:, :, j0 : j0 + DH],
                            )

    return nc


def _get_program():
    global _PROGRAM
    if _PROGRAM is None:
        _PROGRAM = _build_program()
    return _PROGRAM


def kernel(x, Wq, bq, Wk, bk, Wv, bv, E):
    global LAST_RESULTS
    x = np.asarray(x, dtype=np.float32)
    Wq = np.asarray(Wq, dtype=np.float32)
    bq = np.asarray(bq, dtype=np.float32)
    Wk = np.asarray(Wk, dtype=np.float32)
    bk = np.asarray(bk, dtype=np.float32)
    Wv = np.asarray(Wv, dtype=np.float32)
    bv = np.asarray(bv, dtype=np.float32)
    E = np.asarray(E, dtype=np.float32)

    BF = ml_dtypes.bfloat16
    scale = 1.0 / math.sqrt(DH)
    # [d, l] -> [lc, lt, pi, dc, ll] (d = dc*128 + pi, l = lc*512 + lt*128 + ll)
    xTs = [
        np.ascontiguousarray(
            x[b].T.reshape(DC, P, NLC, LT4, P).transpose(2, 3, 1, 0, 4).astype(BF)
        )
        for b in range(B)
    ]
    in_maps = []
    for core in range(NCORES):
        b = core % B
        hg = core // B
        js = slice(hg * J, (hg + 1) * J)
        hs = slice(hg * H, (hg + 1) * H)
        # W.T [D, J] -> [pi, dc, j] so per-partition lines are contiguous
        wqPs = np.ascontiguousarray(
            (Wq[js, :] * scale).T.reshape(DC, P, J).transpose(1, 0, 2).astype(BF)
        )
        wkPs = np.ascontiguousarray(
            Wk[js, :].T.reshape(DC, P, J).transpose(1, 0, 2).astype(BF)
        )
        wvPs = np.ascontiguousarray(
            Wv[js, :].T.reshape(DC, P, J).transpose(1, 0, 2).astype(BF)
        )
        bqTs = np.ascontiguousarray((bq[js] * scale).reshape(JT, P).T)
        bkBs = np.ascontiguousarray(np.broadcast_to(bk[js], (P, J)).astype(BF))
        bvBs = np.ascontiguousarray(np.broadcast_to(bv[js], (P, J)).astype(BF))
        E_s = E[hs]  # [H, KK, L]
        eTs = np.ascontiguousarray(
            E_s.reshape(H, KK, NLC, LT4, P).transpose(2, 4, 0, 3, 1).astype(BF)
        )  # [NLC, P, H, LT4, KK]
        in_maps.append(
            {
                "xT": xTs[b],
                "wqP": wqPs,
                "wkP": wkPs,
                "wvP": wvPs,
                "bqT": bqTs,
                "bkB": bkBs,
                "bvB": bvBs,
                "eT": eTs,
            }
        )

    nc = _get_program()
    res = run_bass_kernel_spmd(nc, in_maps, list(range(NCORES)), trace=TRACE)
    LAST_RESULTS = res

    outp = np.empty((B, L, D), dtype=np.float32)
    for core in range(NCORES):
        b = core % B
        hg = core // B
        outp[b, :, hg * J : (hg + 1) * J] = res.results[core]["out"]
    return outp


# revision 7
# speedup vs baseline: 1.1351x; 1.1351x over previous
"""Linformer attention TRN2 Bass kernel.

Problem: nn_LinformerAttention (B=4, L=4096, D=1024, NH=16, DH=64, k=128).

Sharding: 8 cores = batch(4) x head-group(2). Core c handles batch c%4 and
heads (c//4)*8 .. +8, producing out[b, :, hg*512:(hg+1)*512]. Slices are
disjoint -> no collectives; host reassembles.

All matmul operands are bf16 (fp32 PSUM accumulation). On TRN2, fp32r
matmuls run in fp32_mode=HIGH at ~2 cycles/row and fp32 matmuls run
two-pass LOW_HIGH at 4+ cycles/row; bf16 streams 1 row/cycle. rel-err
budget is 2e-2 and bf16 lands ~6e-3, so this halves PE time for free.
fp8 was measured (host-sim) at 5.8e-2 even with power-of-2 pre-scaling
to dodge denormals — over budget, rejected.

Device algorithm per core, two passes over 8 l-chunks of 512:
  pass A (per chunk): K = x @ Wk.T + bk and V likewise (PSUM accum over 8
    d-subtiles, all K matmuls before all V matmuls so startup only gates
    on Wk; bias+cast to bf16 on DVE); KVp[h] += E_h-chunk.T @ [K|V]
    (Linformer projection accumulated in SBUF, stored per head-PAIR so one
    PE transpose per pair puts odd heads' Kp.T rows at partitions 64..127).
  between passes: KpT per head into zero-padded [128, kk] tiles (head h at
    partitions (h%2)*64..+64, rest ZERO so the dot matmul can contract the
    full 128-partition Q tile); Vp_aug = [Vp | ones].
  pass B (per chunk; x re-DMA'd — cheaper than keeping Q resident):
    - Q.T-chunk = Wq @ x.T + bq (scale 1/sqrt(dh) folded into Wq/bq on
      host), straight into SBUF, consumed immediately
    - per head (software-pipelined so dot-matmuls stream on PE while exp
      runs on Scalar): dotT = KpT.T @ Q.T-chunk; expT = exp(dotT) (ACT,
      logits are small by construction, exp is safe); Xo_aug[l, lt, 65] =
      expT-tile.T @ Vp_aug into ONE PSUM bank (col 64 = softmax denom);
      one batched reciprocal [128,4] + one broadcast-mult [128,4,64]
    - out DMA per head-pair (quarter chunk) so DMA overlaps compute; the
      final chunk uses per-head DMAs alternating sync/gpsimd queues to
      shorten the drain tail.

Scheduling notes (from perfetto traces): DMA descriptor generation
(DIRECT2D) costs ~0.7us per instruction on the issuing engine queue and
was the startup gate. Weights are host-pre-permuted to [128, dc, j] so
each load is one descriptor per partition, and the initial loads are
spread across queues: Wk/biases on Sync, Wv on Vector, E-chunk0 + Wq on
Scalar, x on GpSimd. psQ draws from the psB pool so psD has 4 dedicated
PSUM banks (dot_h only waits on exp_{h-4}).

Host prep (numpy, outside HW-timed region): x[b].T pre-tiled per (chunk,
l-tile) for parallel-issue DMAs, W slices pre-transposed+pre-permuted
(+1/8 scale on Wq), E head-slices pre-transposed, all cast to bf16 (bq
tile stays fp32).
"""

import sys

sys.path.insert(0, "/opt/trn_rl_repo")

import math
from contextlib import ExitStack

import numpy as np
import ml_dtypes

import json

import concourse.bass as bass
import concourse.bass2jax as bass2jax
import concourse.mybir as mybir
import concourse.tile as tile
from concourse.bass_utils import compile_bir_kernel as _orig_compile_bir_kernel
from concourse.bass_utils import run_bass_kernel_spmd
from concourse.masks import make_identity


def _split_multiwaits(bir_json_bytes):
    """This container's walrus encodes at most ONE sync wait per engine
    instruction ("Too many sync wait commands" otherwise), while Tile emits
    multi-wait instructions. Hoist extra waits onto single-wait
    EventSemaphore carrier instructions placed just before, on the same
    engine queue — semantically identical stalling."""
    bj = json.loads(bir_json_bytes)
    for fn in bj["functions"]:
        for blk in fn["blocks"]:
            out = []
            for inst in blk["instructions"]:
                si = inst.get("sync_info")
                waits = (si or {}).get("on_wait") or []
                if si and len(waits) > 1:
                    for wi, w in enumerate(waits[:-1]):
                        out.append(
                            {
                                "debug": inst.get("debug", 0),
                                "engine": inst.get("engine"),
                                "ins": [],
                                "outs": [],
                                "name": inst["name"] + "-w%d" % wi,
                                "opcode": "EventSemaphore",
                                "sync_info": {"on_update": [], "on_wait": [w]},
                            }
                        )
                    si["on_wait"] = [waits[-1]]
                out.append(inst)
            blk["instructions"] = out
    return json.dumps(bj).encode()


def _patched_compile_bir_kernel(bir_json, tmpdir, neff_name="file.neff"):
    return _orig_compile_bir_kernel(_split_multiwaits(bir_json), tmpdir, neff_name)


bass2jax.compile_bir_kernel = _patched_compile_bir_kernel

B, L, D = 4, 4096, 1024
NH, DH, KK = 16, 64, 128
NCORES = 8
HGS = 2  # head groups
H = NH // HGS  # 8 local heads per core
J = H * DH  # 512 output columns per core
P = 128
LCH = 512  # l-chunk
NLC = L // LCH  # 8
DC = D // P  # 8 contraction subtiles
JT = J // P  # 4
LT4 = LCH // P  # 4 l-tiles per chunk
F32 = mybir.dt.float32
BF16 = mybir.dt.bfloat16

TRACE = False  # test.py sets True to collect a profile
LAST_RESULTS = None  # BassKernelResults of the last kernel() call

_PROGRAM = None


def _build_program():
    nc = bass.Bass()
    # x pre-tiled on host: [lc, lt, pi, dc, ll] so each (lc, lt) piece is one
    # DMA with 2 KiB/partition contiguous lines, and pieces spread across
    # DMA queues (the single-queue 1 MiB chunk DMA was gating startup).
    xT = nc.declare_dram_parameter("xT", [NLC, LCH // P, P, D // P, P], BF16, isOutput=False)
    # weights pre-permuted on host to [pi, dc, j]: contiguous per-partition
    # lines -> one descriptor per partition per DMA (cheap DIRECT2D gen).
    wqP = nc.declare_dram_parameter("wqP", [P, DC, J], BF16, isOutput=False)
    wkP = nc.declare_dram_parameter("wkP", [P, DC, J], BF16, isOutput=False)
    wvP = nc.declare_dram_parameter("wvP", [P, DC, J], BF16, isOutput=False)
    bqT = nc.declare_dram_parameter("bqT", [P, JT], F32, isOutput=False)
    bkB = nc.declare_dram_parameter("bkB", [P, J], BF16, isOutput=False)
    bvB = nc.declare_dram_parameter("bvB", [P, J], BF16, isOutput=False)
    eT = nc.declare_dram_parameter("eT", [NLC, P, H, LT4, KK], BF16, isOutput=False)
    out = nc.declare_dram_parameter("out", [L, J], F32, isOutput=True)

    add = mybir.AluOpType.add
    mult = mybir.AluOpType.mult

    with tile.TileContext(nc) as tc:
        with ExitStack() as ctx:
            const = ctx.enter_context(tc.tile_pool(name="const", bufs=1))
            xpool = ctx.enter_context(tc.tile_pool(name="x", bufs=2))
            kvpool = ctx.enter_context(tc.tile_pool(name="kv", bufs=8))
            epool = ctx.enter_context(tc.tile_pool(name="e", bufs=2))
            qtpool = ctx.enter_context(tc.tile_pool(name="qt", bufs=2))
            exppool = ctx.enter_context(tc.tile_pool(name="ex", bufs=4))
            outpool = ctx.enter_context(tc.tile_pool(name="ot", bufs=2))
            recpool = ctx.enter_context(tc.tile_pool(name="rc", bufs=4))
            psA = ctx.enter_context(tc.tile_pool(name="psA", bufs=4, space="PSUM"))
            psB = ctx.enter_context(tc.tile_pool(name="psB", bufs=2, space="PSUM"))
            psXp = ctx.enter_context(tc.tile_pool(name="psX", bufs=2, space="PSUM"))

            # ---- constants resident in SBUF
            wq_sb = const.tile([P, DC, J], BF16, tag="wq")
            wk_sb = const.tile([P, DC, J], BF16, tag="wk")
            wv_sb = const.tile([P, DC, J], BF16, tag="wv")
            # dc=0 slices land first so the first projection matmuls start
            # a fraction of a weight-load into the kernel instead of waiting
            # for the full 1 MiB per weight. Wk on the Sync queue, Wv on the
            # Scalar queue so descriptor generation runs in parallel (DMAs
            # can only issue from the Sync/Scalar/GpSimd queues).
            nc.sync.dma_start(wk_sb[:, 0:1, :], wkP[:, 0:1, :])
            nc.sync.dma_start(wk_sb[:, 1:DC, :], wkP[:, 1:DC, :])
            nc.scalar.dma_start(wv_sb[:, 0:1, :], wvP[:, 0:1, :])
            nc.scalar.dma_start(wv_sb[:, 1:DC, :], wvP[:, 1:DC, :])
            bqT_sb = const.tile([P, JT], F32, tag="bqT")
            bkB_sb = const.tile([P, J], BF16, tag="bkB")
            bvB_sb = const.tile([P, J], BF16, tag="bvB")
            nc.sync.dma_start(bkB_sb[:], bkB[:, :])
            nc.sync.dma_start(bvB_sb[:], bvB[:, :])
            ident = const.tile([P, P], F32, tag="ident")
            make_identity(nc, ident[:])

            # K/V Linformer accumulators, one per head PAIR: [kk, {K,V}, dh-pair]
            kvpP = [const.tile([P, 2, P], F32, tag=f"kvp{t}", name=f"kvp{t}") for t in range(JT)]
            # per-head Kp.T for the dot matmul: head h occupies partitions
            # (h%2)*64..+64, the other 64 partitions are ZERO so the matmul can
            # contract all 128 partitions of the shared Q tile.
            kpT = [const.tile([P, KK], BF16, tag=f"kpT{h}", name=f"kpT{h}") for h in range(H)]
            vpa = [const.tile([P, DH + 1], BF16, tag=f"vpa{h}", name=f"vpa{h}") for h in range(H)]
            for h in range(H):
                b0z = ((h + 1) % 2) * DH  # the half that must stay zero
                nc.vector.memset(kpT[h][b0z : b0z + DH, :], 0.0)

            outr = out[:, :].rearrange("(lo li) j -> li lo j", li=P)

            # ---- pass A: K/V projections + Linformer reduction
            for lc in range(NLC):
                x_sb = xpool.tile([P, LT4, DC, P], BF16, tag="x")
                if lc == 0:
                    # dc=0 sliver of the first l-tile first: the very first
                    # matmul then gates on 32 KiB, not 256 KiB.
                    nc.gpsimd.dma_start(x_sb[:, 0, 0:1, :], xT[0, 0, :, 0:1, :])
                    nc.gpsimd.dma_start(x_sb[:, 0, 1:DC, :], xT[0, 0, :, 1:DC, :])
                    for lt in range(1, LT4):
                        nc.gpsimd.dma_start(x_sb[:, lt, :, :], xT[lc, lt])
                    # E for chunk 0 on the otherwise-idle Scalar queue so it
                    # overlaps the weight loads on Sync/Vector.
                    e_sb0 = epool.tile([P, H, LT4, KK], BF16, tag="e")
                    nc.scalar.dma_start(e_sb0[:, 0 : H // 2], eT[0, :, 0 : H // 2])
                    nc.scalar.dma_start(e_sb0[:, H // 2 : H], eT[0, :, H // 2 : H])
                else:
                    for lt in range(LT4):
                        nc.gpsimd.dma_start(x_sb[:, lt, :, :], xT[lc, lt])
                kv_tiles = []
                for lt in range(LT4):
                    psK = psA.tile([P, LCH], F32, tag="big")
                    psV = psA.tile([P, LCH], F32, tag="big")
                    # K/V interleaved per dc: back-to-back matmuls accumulating
                    # into the SAME PSUM bank stall the PE ~70ns each;
                    # alternating the two banks hides the bubble.
                    for dc in range(DC):
                        nc.tensor.matmul(
                            psK[:], x_sb[:, lt, dc, :],
                            wk_sb[:, dc, :],
                            start=(dc == 0), stop=(dc == DC - 1),
                        )
                        nc.tensor.matmul(
                            psV[:], x_sb[:, lt, dc, :],
                            wv_sb[:, dc, :],
                            start=(dc == 0), stop=(dc == DC - 1),
                        )
                    kv_sb = kvpool.tile([P, 2, LCH], BF16, tag="kv")
                    nc.any.tensor_tensor(kv_sb[:, 0, :], psK[:], bkB_sb[:], add)
                    nc.any.tensor_tensor(kv_sb[:, 1, :], psV[:], bvB_sb[:], add)
                    kv_tiles.append(kv_sb)
                if lc == 0:
                    e_sb = e_sb0
                else:
                    e_sb = epool.tile([P, H, LT4, KK], BF16, tag="e")
                    nc.sync.dma_start(e_sb[:, 0 : H // 2], eT[lc, :, 0 : H // 2])
                    nc.sync.dma_start(e_sb[:, H // 2 : H], eT[lc, :, H // 2 : H])
                # Linformer reduction, head pairs interleaved so the two
                # accumulating PSUM banks alternate (hides the same-bank
                # back-to-back matmul bubble).
                for h0 in range(0, H, 2):
                    psKVs = [psB.tile([P, 2, DH], F32, tag="big", name=f"psKV{i}") for i in range(2)]
                    for lt in range(LT4):
                        for i in range(2):
                            h = h0 + i
                            nc.tensor.matmul(
                                psKVs[i][:], e_sb[:, h, lt, :],
                                kv_tiles[lt][:, :, h * DH : (h + 1) * DH],
                                start=(lt == 0), stop=(lt == LT4 - 1),
                            )
                    for i in range(2):
                        h = h0 + i
                        par = h % 2
                        acc = kvpP[h // 2][:, :, par * DH : (par + 1) * DH]
                        if lc == 0:
                            nc.any.tensor_copy(acc, psKVs[i][:])
                        else:
                            nc.any.tensor_tensor(acc, acc, psKVs[i][:], add)
                if lc == 2:
                    # wq / bqT are first needed in pass B (~t+120us); issue on
                    # the Scalar queue (idle in pass A), deferred past the
                    # startup crunch so the 1 MiB transfer doesn't steal HBM
                    # bandwidth from the pass-A-critical loads.
                    nc.scalar.dma_start(wq_sb[:], wqP[:, :, :])
                    nc.scalar.dma_start(bqT_sb[:], bqT[:, :])

            # ---- between passes: Kp.T / Vp_aug staging
            for t in range(JT):
                # transpose both heads of the pair at once: [kk, dh2] -> [dh2, kk];
                # odd head's rows land at partitions 64..127 by construction
                psT = psB.tile([P, KK], F32, tag="big")
                nc.tensor.transpose(psT[:], kvpP[t][:, 0, :], ident[:])
                for par in range(2):
                    h = 2 * t + par
                    b0 = par * DH
                    nc.any.tensor_copy(kpT[h][b0 : b0 + DH, :], psT[b0 : b0 + DH, :])
                    nc.any.tensor_copy(
                        vpa[h][:, 0:DH], kvpP[t][:, 1, b0 : b0 + DH]
                    )
                    nc.vector.memset(vpa[h][:, DH : DH + 1], 1.0)

            # ---- pass B: Q projection fused with attention, per chunk
            DEPTH = 3  # psD/exp issued this many heads ahead of psX
            for lc in range(NLC):
                x_sb = xpool.tile([P, LT4, DC, P], BF16, tag="x")
                for lt in range(LT4):
                    nc.gpsimd.dma_start(x_sb[:, lt, :, :], xT[lc, lt])
                qt = qtpool.tile([P, JT, LCH], BF16, tag="qt")
                # jt-pairs interleaved across the two psB banks (same-bank
                # accumulation bubble, as above). psQ draws from psB so psD
                # keeps all 4 psA banks: dot_h then only waits on exp_{h-4}.
                for jt0 in (0, 2):
                    psQs = [psB.tile([P, LCH], F32, tag="big", name=f"psQ{i}") for i in range(2)]
                    for dc in range(DC):
                        for i in range(2):
                            jt = jt0 + i
                            nc.tensor.matmul(
                                psQs[i][:], wq_sb[:, dc, jt * P : (jt + 1) * P],
                                x_sb[:, :, dc, :],
                                start=(dc == 0), stop=(dc == DC - 1),
                            )
                    for i in range(2):
                        jt = jt0 + i
                        nc.vector.tensor_scalar(
                            qt[:, jt, :], psQs[i][:], bqT_sb[:, jt : jt + 1], None, add
                        )
                ot = outpool.tile([P, LT4, J], F32, tag="ot")
                exs = [None] * H
                for hh in range(H + DEPTH):
                    if hh < H:
                        h = hh
                        psD = psA.tile([P, LCH], F32, tag="big")
                        nc.tensor.matmul(
                            psD[:], kpT[h][:],
                            qt[:, h // 2, :],
                            start=True, stop=True,
                        )
                        ex = exppool.tile([P, LCH], BF16, tag="ex")
                        nc.scalar.activation(
                            ex[:], psD[:], mybir.ActivationFunctionType.Exp
                        )
                        exs[h] = ex
                    if hh >= DEPTH:
                        h = hh - DEPTH
                        ex = exs[h]
                        psX = psXp.tile([P, LT4, DH + 1], F32, tag="x4")
                        for lt in range(LT4):
                            nc.tensor.matmul(
                                psX[:, lt, :], ex[:, lt * P : (lt + 1) * P],
                                vpa[h][:],
                                start=True, stop=True,
                            )
                        rc = recpool.tile([P, LT4, 1], F32, tag="rc")
                        nc.vector.reciprocal(rc[:], psX[:, :, DH : DH + 1])
                        nc.vector.tensor_tensor(
                            ot[:, :, h * DH : (h + 1) * DH],
                            psX[:, :, 0:DH],
                            rc[:].to_broadcast([P, LT4, DH]),
                            mult,
                        )
                        if lc < NLC - 1:
                            if h % 2 == 1:
                                j0 = (h - 1) * DH
                                nc.sync.dma_start(
                                    outr[:, lc * LT4 : (lc + 1) * LT4, j0 : j0 + 2 * DH],
                                    ot[:, :, j0 : j0 + 2 * DH],
                                )
                        else:
                            # final chunk: per-head DMAs on alternating queues
                            # so the drain tail is as short as possible
                            j0 = h * DH
                            eng = nc.gpsimd if h % 2 == 0 else nc.sync
                            eng.dma_start(
                                outr[:, lc * LT4 : (lc + 1) * LT4, j0 : j0 + DH],
                                ot[:, :, j0 : j0 + DH],
                            )

    return nc


def _get_program():
    global _PROGRAM
    if _PROGRAM is None:
        _PROGRAM = _build_program()
    return _PROGRAM


def kernel(x, Wq, bq, Wk, bk, Wv, bv, E):
    global LAST_RESULTS
    x = np.asarray(x, dtype=np.float32)
    Wq = np.asarray(Wq, dtype=np.float32)
    bq = np.asarray(bq, dtype=np.float32)
    Wk = np.asarray(Wk, dtype=np.float32)
    bk = np.asarray(bk, dtype=np.float32)
    Wv = np.asarray(Wv, dtype=np.float32)
    bv = np.asarray(bv, dtype=np.float32)
    E = np.asarray(E, dtype=np.float32)

    BF = ml_dtypes.bfloat16
    scale = 1.0 / math.sqrt(DH)
    # [d, l] -> [lc, lt, pi, dc, ll] (d = dc*128 + pi, l = lc*512 + lt*128 + ll)
    xTs = [
        np.ascontiguousarray(
            x[b].T.reshape(DC, P, NLC, LT4, P).transpose(2, 3, 1, 0, 4).astype(BF)
        )
        for b in range(B)
    ]
    in_maps = []
    for core in range(NCORES):
        b = core % B
        hg = core // B
        js = slice(hg * J, (hg + 1) * J)
        hs = slice(hg * H, (hg + 1) * H)
        # W.T [D, J] -> [pi, dc, j] so per-partition lines are contiguous
        wqPs = np.ascontiguousarray(
            (Wq[js, :] * scale).T.reshape(DC, P, J).transpose(1, 0, 2).astype(BF)
        )
        wkPs = np.ascontiguousarray(
            Wk[js, :].T.reshape(DC, P, J).transpose(1, 0, 2).astype(BF)
        )
        wvPs = np.ascontiguousarray(
            Wv[js, :].T.reshape(DC, P, J).transpose(1, 0, 2).astype(BF)
        )
        bqTs = np.ascontiguousarray((bq[js] * scale).reshape(JT, P).T)
        bkBs = np.ascontiguousarray(np.broadcast_to(bk[js], (P, J)).astype(BF))
        bvBs = np.ascontiguousarray(np.broadcast_to(bv[js], (P, J)).astype(BF))
        E_s = E[hs]  # [H, KK, L]
        eTs = np.ascontiguousarray(
            E_s.reshape(H, KK, NLC, LT4, P).transpose(2, 4, 0, 3, 1).astype(BF)
        )  # [NLC, P, H, LT4, KK]
        in_maps.append(
            {
                "xT": xTs[b],
                "wqP": wqPs,
                "wkP": wkPs,
                "wvP": wvPs,
                "bqT": bqTs,
                "bkB": bkBs,
                "bvB": bvBs,
                "eT": eTs,
            }
        )

    nc = _get_program()
    res = run_bass_kernel_spmd(nc, in_maps, list(range(NCORES)), trace=TRACE)
    LAST_RESULTS = res

    outp = np.empty((B, L, D), dtype=np.float32)
    for core in range(NCORES):
        b = core % B
        hg = core // B
        outp[b, :, hg * J : (hg + 1) * J] = res.results[core]["out"]
    return outp


# revision 9
# speedup vs baseline: 1.1768x; 1.0367x over previous
"""Linformer attention TRN2 Bass kernel.

Problem: nn_LinformerAttention (B=4, L=4096, D=1024, NH=16, DH=64, k=128).

Sharding: 8 cores = batch(4) x head-group(2). Core c handles batch c%4 and
heads (c//4)*8 .. +8, producing out[b, :, hg*512:(hg+1)*512]. Slices are
disjoint -> no collectives; host reassembles.

All matmul operands are bf16 (fp32 PSUM accumulation). On TRN2, fp32r
matmuls run in fp32_mode=HIGH at ~2 cycles/row and fp32 matmuls run
two-pass LOW_HIGH at 4+ cycles/row; bf16 streams 1 row/cycle. rel-err
budget is 2e-2 and bf16 lands ~6e-3, so this halves PE time for free.
fp8 was measured (host-sim) at 5.8e-2 even with power-of-2 pre-scaling
to dodge denormals — over budget, rejected.

Device algorithm per core, two passes over 8 l-chunks of 512:
  pass A (per chunk): K = x @ Wk.T + bk and V likewise (PSUM accum over 8
    d-subtiles, all K matmuls before all V matmuls so startup only gates
    on Wk; bias+cast to bf16 on DVE); KVp[h] += E_h-chunk.T @ [K|V]
    (Linformer projection accumulated in SBUF, stored per head-PAIR so one
    PE transpose per pair puts odd heads' Kp.T rows at partitions 64..127).
  between passes: KpT per head into zero-padded [128, kk] tiles (head h at
    partitions (h%2)*64..+64, rest ZERO so the dot matmul can contract the
    full 128-partition Q tile); Vp_aug = [Vp | ones].
  pass B (per chunk; x re-DMA'd — cheaper than keeping Q resident):
    - Q.T-chunk = Wq @ x.T + bq (scale 1/sqrt(dh) folded into Wq/bq on
      host), straight into SBUF, consumed immediately
    - per head (software-pipelined so dot-matmuls stream on PE while exp
      runs on Scalar): dotT = KpT.T @ Q.T-chunk; expT = exp(dotT) (ACT,
      logits are small by construction, exp is safe); Xo_aug[l, lt, 65] =
      expT-tile.T @ Vp_aug into ONE PSUM bank (col 64 = softmax denom);
      one batched reciprocal [128,4] + one broadcast-mult [128,4,64]
    - out DMA per head-pair (quarter chunk) so DMA overlaps compute; the
      final chunk uses per-head DMAs alternating sync/gpsimd queues to
      shorten the drain tail.

Scheduling notes (from perfetto traces): DMA descriptor generation
(DIRECT2D) costs ~0.7us per instruction on the issuing engine queue and
was the startup gate. Weights are host-pre-permuted to [128, dc, j] so
each load is one descriptor per partition, and the initial loads are
spread across queues: Wk/biases on Sync, Wv on Vector, E-chunk0 + Wq on
Scalar, x on GpSimd. psQ draws from the psB pool so psD has 4 dedicated
PSUM banks (dot_h only waits on exp_{h-4}).

Host prep (numpy, outside HW-timed region): x[b].T pre-tiled per (chunk,
l-tile) for parallel-issue DMAs, W slices pre-transposed+pre-permuted
(+1/8 scale on Wq), E head-slices pre-transposed, all cast to bf16 (bq
tile stays fp32).
"""

import sys

sys.path.insert(0, "/opt/trn_rl_repo")

import math
from contextlib import ExitStack

import numpy as np
import ml_dtypes

import json

import concourse.bass as bass
import concourse.bass2jax as bass2jax
import concourse.mybir as mybir
import concourse.tile as tile
from concourse.bass_utils import compile_bir_kernel as _orig_compile_bir_kernel
from concourse.bass_utils import run_bass_kernel_spmd
from concourse.masks import make_identity


def _split_multiwaits(bir_json_bytes):
    """This container's walrus encodes at most ONE sync wait per engine
    instruction ("Too many sync wait commands" otherwise), while Tile emits
    multi-wait instructions. Hoist extra waits onto single-wait
    EventSemaphore carrier instructions placed just before, on the same
    engine queue — semantically identical stalling."""
    bj = json.loads(bir_json_bytes)
    for fn in bj["functions"]:
        for blk in fn["blocks"]:
            out = []
            for inst in blk["instructions"]:
                si = inst.get("sync_info")
                waits = (si or {}).get("on_wait") or []
                if si and len(waits) > 1:
                    for wi, w in enumerate(waits[:-1]):
                        out.append(
                            {
                                "debug": inst.get("debug", 0),
                                "engine": inst.get("engine"),
                                "ins": [],
                                "outs": [],
                                "name": inst["name"] + "-w%d" % wi,
                                "opcode": "EventSemaphore",
                                "sync_info": {"on_update": [], "on_wait": [w]},
                            }
                        )
                    si["on_wait"] = [waits[-1]]
                out.append(inst)
            blk["instructions"] = out
    return json.dumps(bj).encode()


def _patched_compile_bir_kernel(bir_json, tmpdir, neff_name="file.neff"):
    return _orig_compile_bir_kernel(_split_multiwaits(bir_json), tmpdir, neff_name)


bass2jax.compile_bir_kernel = _patched_compile_bir_kernel

B, L, D = 4, 4096, 1024
NH, DH, KK = 16, 64, 128
NCORES = 8
HGS = 2  # head groups
H = NH // HGS  # 8 local heads per core
J = H * DH  # 512 output columns per core
P = 128
LCH = 512  # l-chunk
NLC = L // LCH  # 8
DC = D // P  # 8 contraction subtiles
JT = J // P  # 4
LT4 = LCH // P  # 4 l-tiles per chunk
F32 = mybir.dt.float32
BF16 = mybir.dt.bfloat16

TRACE = False  # test.py sets True to collect a profile
LAST_RESULTS = None  # BassKernelResults of the last kernel() call

_PROGRAM = None


def _build_program():
    nc = bass.Bass()
    # x pre-tiled on host: [lc, lt, pi, dc, ll] so each (lc, lt) piece is one
    # DMA with 2 KiB/partition contiguous lines, and pieces spread across
    # DMA queues (the single-queue 1 MiB chunk DMA was gating startup).
    xT = nc.declare_dram_parameter("xT", [NLC, LCH // P, P, D // P, P], BF16, isOutput=False)
    # weights pre-permuted on host to [pi, dc, j]: contiguous per-partition
    # lines -> one descriptor per partition per DMA (cheap DIRECT2D gen).
    wqP = nc.declare_dram_parameter("wqP", [P, DC, J], BF16, isOutput=False)
    wkP = nc.declare_dram_parameter("wkP", [P, DC, J], BF16, isOutput=False)
    wvP = nc.declare_dram_parameter("wvP", [P, DC, J], BF16, isOutput=False)
    bqT = nc.declare_dram_parameter("bqT", [P, JT], F32, isOutput=False)
    bkB = nc.declare_dram_parameter("bkB", [P, J], BF16, isOutput=False)
    bvB = nc.declare_dram_parameter("bvB", [P, J], BF16, isOutput=False)
    eT = nc.declare_dram_parameter("eT", [NLC, P, H, LT4, KK], BF16, isOutput=False)
    out = nc.declare_dram_parameter("out", [L, J], F32, isOutput=True)

    add = mybir.AluOpType.add
    mult = mybir.AluOpType.mult

    with tile.TileContext(nc) as tc:
        with ExitStack() as ctx:
            const = ctx.enter_context(tc.tile_pool(name="const", bufs=1))
            xpool = ctx.enter_context(tc.tile_pool(name="x", bufs=2))
            kvpool = ctx.enter_context(tc.tile_pool(name="kv", bufs=8))
            epool = ctx.enter_context(tc.tile_pool(name="e", bufs=2))
            qtpool = ctx.enter_context(tc.tile_pool(name="qt", bufs=2))
            exppool = ctx.enter_context(tc.tile_pool(name="ex", bufs=4))
            outpool = ctx.enter_context(tc.tile_pool(name="ot", bufs=2))
            recpool = ctx.enter_context(tc.tile_pool(name="rc", bufs=4))
            psA = ctx.enter_context(tc.tile_pool(name="psA", bufs=4, space="PSUM"))
            psB = ctx.enter_context(tc.tile_pool(name="psB", bufs=2, space="PSUM"))
            psXp = ctx.enter_context(tc.tile_pool(name="psX", bufs=2, space="PSUM"))

            # ---- constants resident in SBUF
            wq_sb = const.tile([P, DC, J], BF16, tag="wq")
            wk_sb = const.tile([P, DC, J], BF16, tag="wk")
            wv_sb = const.tile([P, DC, J], BF16, tag="wv")
            # dc=0 slices land first so the first projection matmuls start
            # a fraction of a weight-load into the kernel instead of waiting
            # for the full 1 MiB per weight. Wk on the Sync queue, Wv on the
            # Scalar queue so descriptor generation runs in parallel (DMAs
            # can only issue from the Sync/Scalar/GpSimd queues).
            nc.sync.dma_start(wk_sb[:, 0:1, :], wkP[:, 0:1, :])
            nc.sync.dma_start(wk_sb[:, 1:DC, :], wkP[:, 1:DC, :])
            nc.scalar.dma_start(wv_sb[:, 0:1, :], wvP[:, 0:1, :])
            nc.scalar.dma_start(wv_sb[:, 1:DC, :], wvP[:, 1:DC, :])
            bqT_sb = const.tile([P, JT], F32, tag="bqT")
            bkB_sb = const.tile([P, J], BF16, tag="bkB")
            bvB_sb = const.tile([P, J], BF16, tag="bvB")
            nc.sync.dma_start(bkB_sb[:], bkB[:, :])
            nc.sync.dma_start(bvB_sb[:], bvB[:, :])
            ident = const.tile([P, P], F32, tag="ident")
            make_identity(nc, ident[:])

            # K/V Linformer accumulators, one per head PAIR: [kk, {K,V}, dh-pair]
            kvpP = [const.tile([P, 2, P], F32, tag=f"kvp{t}", name=f"kvp{t}") for t in range(JT)]
            # per-head Kp.T for the dot matmul: head h occupies partitions
            # (h%2)*64..+64, the other 64 partitions are ZERO so the matmul can
            # contract all 128 partitions of the shared Q tile.
            kpT = [const.tile([P, KK], BF16, tag=f"kpT{h}", name=f"kpT{h}") for h in range(H)]
            vpa = [const.tile([P, DH + 1], BF16, tag=f"vpa{h}", name=f"vpa{h}") for h in range(H)]
            for h in range(H):
                b0z = ((h + 1) % 2) * DH  # the half that must stay zero
                nc.vector.memset(kpT[h][b0z : b0z + DH, :], 0.0)

            outr = out[:, :].rearrange("(lo li) j -> li lo j", li=P)

            # ---- pass A: K/V projections + Linformer reduction
            for lc in range(NLC):
                x_sb = xpool.tile([P, LT4, DC, P], BF16, tag="x")
                for lt in range(LT4):
                    nc.gpsimd.dma_start(x_sb[:, lt, :, :], xT[lc, lt])
                kv_tiles = []
                if lc == 0:
                    # startup: all K matmuls of an lt-PAIR first (interleaved
                    # across the two psK banks so no same-bank bubble), V after
                    # — the PE then only gates on the Wk load, and Wv's 1 MiB
                    # has ~4us of K-work to land under.
                    psKs, psVs, kvs = [], [], []
                    for lt in range(LT4):
                        psKs.append(psA.tile([P, LCH], F32, tag="big", name=f"psK{lt}"))
                        psVs.append(psA.tile([P, LCH], F32, tag="big", name=f"psV{lt}"))
                        kvs.append(kvpool.tile([P, 2, LCH], BF16, tag="kv", name=f"kv{lt}"))
                    for lt0 in (0, 2):
                        for dc in range(DC):
                            for lt in (lt0, lt0 + 1):
                                nc.tensor.matmul(
                                    psKs[lt][:], x_sb[:, lt, dc, :],
                                    wk_sb[:, dc, :],
                                    start=(dc == 0), stop=(dc == DC - 1),
                                )
                        for dc in range(DC):
                            for lt in (lt0, lt0 + 1):
                                nc.tensor.matmul(
                                    psVs[lt][:], x_sb[:, lt, dc, :],
                                    wv_sb[:, dc, :],
                                    start=(dc == 0), stop=(dc == DC - 1),
                                )
                        for lt in (lt0, lt0 + 1):
                            nc.any.tensor_tensor(kvs[lt][:, 0, :], psKs[lt][:], bkB_sb[:], add)
                            nc.any.tensor_tensor(kvs[lt][:, 1, :], psVs[lt][:], bvB_sb[:], add)
                    kv_tiles = kvs
                else:
                    for lt in range(LT4):
                        psK = psA.tile([P, LCH], F32, tag="big")
                        psV = psA.tile([P, LCH], F32, tag="big")
                        # K/V interleaved per dc: alternating the two PSUM banks
                        # hides the same-bank accumulation bubble.
                        for dc in range(DC):
                            nc.tensor.matmul(
                                psK[:], x_sb[:, lt, dc, :],
                                wk_sb[:, dc, :],
                                start=(dc == 0), stop=(dc == DC - 1),
                            )
                            nc.tensor.matmul(
                                psV[:], x_sb[:, lt, dc, :],
                                wv_sb[:, dc, :],
                                start=(dc == 0), stop=(dc == DC - 1),
                            )
                        kv_sb = kvpool.tile([P, 2, LCH], BF16, tag="kv")
                        nc.any.tensor_tensor(kv_sb[:, 0, :], psK[:], bkB_sb[:], add)
                        nc.any.tensor_tensor(kv_sb[:, 1, :], psV[:], bvB_sb[:], add)
                        kv_tiles.append(kv_sb)
                e_sb = epool.tile([P, H, LT4, KK], BF16, tag="e")
                nc.sync.dma_start(e_sb[:, 0 : H // 2], eT[lc, :, 0 : H // 2])
                nc.sync.dma_start(e_sb[:, H // 2 : H], eT[lc, :, H // 2 : H])
                for h in range(H):
                    par = h % 2
                    acc = kvpP[h // 2][:, :, par * DH : (par + 1) * DH]
                    psKV = psB.tile([P, 2, DH], F32, tag="big")
                    for lt in range(LT4):
                        nc.tensor.matmul(
                            psKV[:], e_sb[:, h, lt, :],
                            kv_tiles[lt][:, :, h * DH : (h + 1) * DH],
                            start=(lt == 0), stop=(lt == LT4 - 1),
                        )
                    if lc == 0:
                        nc.any.tensor_copy(acc, psKV[:])
                    else:
                        nc.any.tensor_tensor(acc, acc, psKV[:], add)
                if lc == 2:
                    # wq / bqT are first needed in pass B (~t+120us); issue on
                    # the Scalar queue (idle in pass A), deferred past the
                    # startup crunch so the 1 MiB transfer doesn't steal HBM
                    # bandwidth from the pass-A-critical loads.
                    nc.scalar.dma_start(wq_sb[:], wqP[:, :, :])
                    nc.scalar.dma_start(bqT_sb[:], bqT[:, :])

            # ---- between passes: Kp.T / Vp_aug staging
            for t in range(JT):
                # transpose both heads of the pair at once: [kk, dh2] -> [dh2, kk];
                # odd head's rows land at partitions 64..127 by construction
                psT = psB.tile([P, KK], F32, tag="big")
                nc.tensor.transpose(psT[:], kvpP[t][:, 0, :], ident[:])
                for par in range(2):
                    h = 2 * t + par
                    b0 = par * DH
                    nc.any.tensor_copy(kpT[h][b0 : b0 + DH, :], psT[b0 : b0 + DH, :])
                    nc.any.tensor_copy(
                        vpa[h][:, 0:DH], kvpP[t][:, 1, b0 : b0 + DH]
                    )
                    nc.vector.memset(vpa[h][:, DH : DH + 1], 1.0)

            # ---- pass B: Q projection fused with attention, per chunk
            DEPTH = 3  # psD/exp issued this many heads ahead of psX
            for lc in range(NLC):
                x_sb = xpool.tile([P, LT4, DC, P], BF16, tag="x")
                for lt in range(LT4):
                    nc.gpsimd.dma_start(x_sb[:, lt, :, :], xT[lc, lt])
                qt = qtpool.tile([P, JT, LCH], BF16, tag="qt")
                # jt-pairs interleaved across the two psB banks (same-bank
                # accumulation bubble, as above). psQ draws from psB so psD
                # keeps all 4 psA banks: dot_h then only waits on exp_{h-4}.
                for jt0 in (0, 2):
                    psQs = [psB.tile([P, LCH], F32, tag="big", name=f"psQ{i}") for i in range(2)]
                    for dc in range(DC):
                        for i in range(2):
                            jt = jt0 + i
                            nc.tensor.matmul(
                                psQs[i][:], wq_sb[:, dc, jt * P : (jt + 1) * P],
                                x_sb[:, :, dc, :],
                                start=(dc == 0), stop=(dc == DC - 1),
                            )
                    for i in range(2):
                        jt = jt0 + i
                        # bias-add on the Scalar engine (ACT: out = f(in*1+bias))
                        # — GpSimd can't read PSUM and the DVE queue would make
                        # psQ bank recycling wait behind the previous chunk's
                        # reciprocal/mult ops.
                        nc.scalar.activation(
                            qt[:, jt, :], psQs[i][:],
                            mybir.ActivationFunctionType.Identity,
                            bias=bqT_sb[:, jt : jt + 1],
                        )
                ot = outpool.tile([P, LT4, J], F32, tag="ot")
                exs = [None] * H
                for hh in range(H + DEPTH):
                    if hh < H:
                        h = hh
                        psD = psA.tile([P, LCH], F32, tag="big")
                        nc.tensor.matmul(
                            psD[:], kpT[h][:],
                            qt[:, h // 2, :],
                            start=True, stop=True,
                        )
                        ex = exppool.tile([P, LCH], BF16, tag="ex")
                        nc.scalar.activation(
                            ex[:], psD[:], mybir.ActivationFunctionType.Exp
                        )
                        exs[h] = ex
                    if hh >= DEPTH:
                        h = hh - DEPTH
                        ex = exs[h]
                        psX = psXp.tile([P, LT4, DH + 1], F32, tag="x4")
                        for lt in range(LT4):
                            nc.tensor.matmul(
                                psX[:, lt, :], ex[:, lt * P : (lt + 1) * P],
                                vpa[h][:],
                                start=True, stop=True,
                            )
                        rc = recpool.tile([P, LT4, 1], F32, tag="rc")
                        nc.vector.reciprocal(rc[:], psX[:, :, DH : DH + 1])
                        nc.vector.tensor_tensor(
                            ot[:, :, h * DH : (h + 1) * DH],
                            psX[:, :, 0:DH],
                            rc[:].to_broadcast([P, LT4, DH]),
                            mult,
                        )
                        if h % 2 == 1:
                            j0 = (h - 1) * DH
                            nc.sync.dma_start(
                                outr[:, lc * LT4 : (lc + 1) * LT4, j0 : j0 + 2 * DH],
                                ot[:, :, j0 : j0 + 2 * DH],
                            )

    return nc


def _get_program():
    global _PROGRAM
    if _PROGRAM is None:
        _PROGRAM = _build_program()
    return _PROGRAM


def kernel(x, Wq, bq, Wk, bk, Wv, bv, E):
    global LAST_RESULTS
    x = np.asarray(x, dtype=np.float32)
    Wq = np.asarray(Wq, dtype=np.float32)
    bq = np.asarray(bq, dtype=np.float32)
    Wk = np.asarray(Wk, dtype=np.float32)
    bk = np.asarray(bk, dtype=np.float32)
    Wv = np.asarray(Wv, dtype=np.float32)
    bv = np.asarray(bv, dtype=np.float32)
    E = np.asarray(E, dtype=np.float32)

    BF = ml_dtypes.bfloat16
    scale = 1.0 / math.sqrt(DH)
    # [d, l] -> [lc, lt, pi, dc, ll] (d = dc*128 + pi, l = lc*512 + lt*128 + ll)
    xTs = [
        np.ascontiguousarray(
            x[b].T.reshape(DC, P, NLC, LT4, P).transpose(2, 3, 1, 0, 4).astype(BF)
        )
        for b in range(B)
    ]
    in_maps = []
    for core in range(NCORES):
        b = core % B
        hg = core // B
        js = slice(hg * J, (hg + 1) * J)
        hs = slice(hg * H, (hg + 1) * H)
        # W.T [D, J] -> [pi, dc, j] so per-partition lines are contiguous
        wqPs = np.ascontiguousarray(
            (Wq[js, :] * scale).T.reshape(DC, P, J).transpose(1, 0, 2).astype(BF)
        )
        wkPs = np.ascontiguousarray(
            Wk[js, :].T.reshape(DC, P, J).transpose(1, 0, 2).astype(BF)
        )
        wvPs = np.ascontiguousarray(
            Wv[js, :].T.reshape(DC, P, J).transpose(1, 0, 2).astype(BF)
        )
        bqTs = np.ascontiguousarray((bq[js] * scale).reshape(JT, P).T)
        bkBs = np.ascontiguousarray(np.broadcast_to(bk[js], (P, J)).astype(BF))
        bvBs = np.ascontiguousarray(np.broadcast_to(bv[js], (P, J)).astype(BF))
        E_s = E[hs]  # [H, KK, L]
        eTs = np.ascontiguousarray(
            E_s.reshape(H, KK, NLC, LT4, P).transpose(2, 4, 0, 3, 1).astype(BF)
        )  # [NLC, P, H, LT4, KK]
        in_maps.append(
            {
                "xT": xTs[b],
                "wqP": wqPs,
                "wkP": wkPs,
                "wvP": wvPs,
                "bqT": bqTs,
                "bkB": bkBs,
                "bvB": bvBs,
                "eT": eTs,
            }
        )

    nc = _get_program()
    res = run_bass_kernel_spmd(nc, in_maps, list(range(NCORES)), trace=TRACE)
    LAST_RESULTS = res

    outp = np.empty((B, L, D), dtype=np.float32)
    for core in range(NCORES):
        b = core % B
        hg = core // B
        outp[b, :, hg * J : (hg + 1) * J] = res.results[core]["out"]
    return outp


# revision 10
# speedup vs baseline: 1.1770x; 1.0002x over previous
"""Linformer attention TRN2 Bass kernel.

Problem: nn_LinformerAttention (B=4, L=4096, D=1024, NH=16, DH=64, k=128).

Sharding: 8 cores = batch(4) x head-group(2). Core c handles batch c%4 and
heads (c//4)*8 .. +8, producing out[b, :, hg*512:(hg+1)*512]. Slices are
disjoint -> no collectives; host reassembles.

All matmul operands are bf16 (fp32 PSUM accumulation). On TRN2, fp32r
matmuls run in fp32_mode=HIGH at ~2 cycles/row and fp32 matmuls run
two-pass LOW_HIGH at 4+ cycles/row; bf16 streams 1 row/cycle. rel-err
budget is 2e-2 and bf16 lands ~6e-3, so this halves PE time for free.
fp8 was measured (host-sim) at 5.8e-2 even with power-of-2 pre-scaling
to dodge denormals — over budget, rejected.

Device algorithm per core, two passes over 8 l-chunks of 512:
  pass A (per chunk): K = x @ Wk.T + bk and V likewise (PSUM accum over 8
    d-subtiles, all K matmuls before all V matmuls so startup only gates
    on Wk; bias+cast to bf16 on DVE); KVp[h] += E_h-chunk.T @ [K|V]
    (Linformer projection accumulated in SBUF, stored per head-PAIR so one
    PE transpose per pair puts odd heads' Kp.T rows at partitions 64..127).
  between passes: KpT per head into zero-padded [128, kk] tiles (head h at
    partitions (h%2)*64..+64, rest ZERO so the dot matmul can contract the
    full 128-partition Q tile); Vp_aug = [Vp | ones].
  pass B (per chunk; x re-DMA'd — cheaper than keeping Q resident):
    - Q.T-chunk = Wq @ x.T + bq (scale 1/sqrt(dh) folded into Wq/bq on
      host), straight into SBUF, consumed immediately
    - per head (software-pipelined so dot-matmuls stream on PE while exp
      runs on Scalar): dotT = KpT.T @ Q.T-chunk; expT = exp(dotT) (ACT,
      logits are small by construction, exp is safe); Xo_aug[l, lt, 65] =
      expT-tile.T @ Vp_aug into ONE PSUM bank (col 64 = softmax denom);
      one batched reciprocal [128,4] + one broadcast-mult [128,4,64]
    - out DMA per head-pair (quarter chunk) so DMA overlaps compute; the
      final chunk uses per-head DMAs alternating sync/gpsimd queues to
      shorten the drain tail.

Scheduling notes (from perfetto traces): DMA descriptor generation
(DIRECT2D) costs ~0.7us per instruction on the issuing engine queue and
was the startup gate. Weights are host-pre-permuted to [128, dc, j] so
each load is one descriptor per partition, and the initial loads are
spread across queues: Wk/biases on Sync, Wv on Vector, E-chunk0 + Wq on
Scalar, x on GpSimd. psQ draws from the psB pool so psD has 4 dedicated
PSUM banks (dot_h only waits on exp_{h-4}).

Host prep (numpy, outside HW-timed region): x[b].T pre-tiled per (chunk,
l-tile) for parallel-issue DMAs, W slices pre-transposed+pre-permuted
(+1/8 scale on Wq), E head-slices pre-transposed, all cast to bf16 (bq
tile stays fp32).
"""

import sys

sys.path.insert(0, "/opt/trn_rl_repo")

import math
from contextlib import ExitStack

import numpy as np
import ml_dtypes

import json

import concourse.bass as bass
import concourse.bass2jax as bass2jax
import concourse.mybir as mybir
import concourse.tile as tile
from concourse.bass_utils import compile_bir_kernel as _orig_compile_bir_kernel
from concourse.bass_utils import run_bass_kernel_spmd
from concourse.masks import make_identity


def _split_multiwaits(bir_json_bytes):
    """This container's walrus encodes at most ONE sync wait per engine
    instruction ("Too many sync wait commands" otherwise), while Tile emits
    multi-wait instructions. Hoist extra waits onto single-wait
    EventSemaphore carrier instructions placed just before, on the same
    engine queue — semantically identical stalling."""
    bj = json.loads(bir_json_bytes)
    for fn in bj["functions"]:
        for blk in fn["blocks"]:
            out = []
            for inst in blk["instructions"]:
                si = inst.get("sync_info")
                waits = (si or {}).get("on_wait") or []
                if si and len(waits) > 1:
                    for wi, w in enumerate(waits[:-1]):
                        out.append(
                            {
                                "debug": inst.get("debug", 0),
                                "engine": inst.get("engine"),
                                "ins": [],
                                "outs": [],
                                "name": inst["name"] + "-w%d" % wi,
                                "opcode": "EventSemaphore",
                                "sync_info": {"on_update": [], "on_wait": [w]},
                            }
                        )
                    si["on_wait"] = [waits[-1]]
                out.append(inst)
            blk["instructions"] = out
    return json.dumps(bj).encode()


def _patched_compile_bir_kernel(bir_json, tmpdir, neff_name="file.neff"):
    return _orig_compile_bir_kernel(_split_multiwaits(bir_json), tmpdir, neff_name)


bass2jax.compile_bir_kernel = _patched_compile_bir_kernel

B, L, D = 4, 4096, 1024
NH, DH, KK = 16, 64, 128
NCORES = 8
HGS = 2  # head groups
H = NH // HGS  # 8 local heads per core
J = H * DH  # 512 output columns per core
P = 128
LCH = 512  # l-chunk
NLC = L // LCH  # 8
DC = D // P  # 8 contraction subtiles
JT = J // P  # 4
LT4 = LCH // P  # 4 l-tiles per chunk
F32 = mybir.dt.float32
BF16 = mybir.dt.bfloat16

TRACE = False  # test.py sets True to collect a profile
LAST_RESULTS = None  # BassKernelResults of the last kernel() call

_PROGRAM = None


def _build_program():
    nc = bass.Bass()
    # x pre-tiled on host: [lc, lt, pi, dc, ll] so each (lc, lt) piece is one
    # DMA with 2 KiB/partition contiguous lines, and pieces spread across
    # DMA queues (the single-queue 1 MiB chunk DMA was gating startup).
    xT = nc.declare_dram_parameter("xT", [NLC, P, DC, LT4, P], BF16, isOutput=False)
    # weights pre-permuted on host to [pi, dc, j]: contiguous per-partition
    # lines -> one descriptor per partition per DMA (cheap DIRECT2D gen).
    wqP = nc.declare_dram_parameter("wqP", [P, DC, J], BF16, isOutput=False)
    wkP = nc.declare_dram_parameter("wkP", [P, DC, J], BF16, isOutput=False)
    wvP = nc.declare_dram_parameter("wvP", [P, DC, J], BF16, isOutput=False)
    bqT = nc.declare_dram_parameter("bqT", [P, JT], F32, isOutput=False)
    bkB = nc.declare_dram_parameter("bkB", [P, J], BF16, isOutput=False)
    bvB = nc.declare_dram_parameter("bvB", [P, J], BF16, isOutput=False)
    eT = nc.declare_dram_parameter("eT", [NLC, P, H, LT4, KK], BF16, isOutput=False)
    out = nc.declare_dram_parameter("out", [L, J], F32, isOutput=True)

    add = mybir.AluOpType.add
    mult = mybir.AluOpType.mult

    with tile.TileContext(nc) as tc:
        with ExitStack() as ctx:
            const = ctx.enter_context(tc.tile_pool(name="const", bufs=1))
            kvpool = ctx.enter_context(tc.tile_pool(name="kv", bufs=8))
            epool = ctx.enter_context(tc.tile_pool(name="e", bufs=2))
            qtpool = ctx.enter_context(tc.tile_pool(name="qt", bufs=2))
            exppool = ctx.enter_context(tc.tile_pool(name="ex", bufs=4))
            outpool = ctx.enter_context(tc.tile_pool(name="ot", bufs=2))
            recpool = ctx.enter_context(tc.tile_pool(name="rc", bufs=4))
            psA = ctx.enter_context(tc.tile_pool(name="psA", bufs=4, space="PSUM"))
            psB = ctx.enter_context(tc.tile_pool(name="psB", bufs=2, space="PSUM"))
            psXp = ctx.enter_context(tc.tile_pool(name="psX", bufs=2, space="PSUM"))

            # ---- constants resident in SBUF
            wq_sb = const.tile([P, DC, J], BF16, tag="wq")
            wk_sb = const.tile([P, DC, J], BF16, tag="wk")
            wv_sb = const.tile([P, DC, J], BF16, tag="wv")
            # dc=0 slices land first so the first projection matmuls start
            # a fraction of a weight-load into the kernel instead of waiting
            # for the full 1 MiB per weight. Wk on the Sync queue, Wv on the
            # Scalar queue so descriptor generation runs in parallel (DMAs
            # can only issue from the Sync/Scalar/GpSimd queues).
            nc.sync.dma_start(wk_sb[:, 0:1, :], wkP[:, 0:1, :])
            nc.sync.dma_start(wk_sb[:, 1:4, :], wkP[:, 1:4, :])
            nc.sync.dma_start(wk_sb[:, 4:DC, :], wkP[:, 4:DC, :])
            nc.scalar.dma_start(wv_sb[:, 0:1, :], wvP[:, 0:1, :])
            nc.scalar.dma_start(wv_sb[:, 1:4, :], wvP[:, 1:4, :])
            nc.scalar.dma_start(wv_sb[:, 4:DC, :], wvP[:, 4:DC, :])
            bqT_sb = const.tile([P, JT], F32, tag="bqT")
            bkB_sb = const.tile([P, J], BF16, tag="bkB")
            bvB_sb = const.tile([P, J], BF16, tag="bvB")
            nc.sync.dma_start(bkB_sb[:], bkB[:, :])
            nc.sync.dma_start(bvB_sb[:], bvB[:, :])
            ident = const.tile([P, P], F32, tag="ident")
            make_identity(nc, ident[:])

            # x resident for the whole kernel: loaded once in pass A, read in
            # place by pass B (no reload, no pass-B x DMAs). Layout [pi, lc,
            # dc, lt, ll]: pass-B's moving operand x[:, lc, dc] is contiguous.
            xs_all = const.tile([P, NLC, DC, LT4, P], BF16, tag="xs")

            # K/V Linformer accumulators, one per head PAIR: [kk, {K,V}, dh-pair]
            kvpP = [const.tile([P, 2, P], F32, tag=f"kvp{t}", name=f"kvp{t}") for t in range(JT)]
            # per-head Kp.T for the dot matmul: head h occupies partitions
            # (h%2)*64..+64, the other 64 partitions are ZERO so the matmul can
            # contract all 128 partitions of the shared Q tile.
            kpT = [const.tile([P, KK], BF16, tag=f"kpT{h}", name=f"kpT{h}") for h in range(H)]
            vpa = [const.tile([P, DH + 1], BF16, tag=f"vpa{h}", name=f"vpa{h}") for h in range(H)]
            for h in range(H):
                b0z = ((h + 1) % 2) * DH  # the half that must stay zero
                nc.vector.memset(kpT[h][b0z : b0z + DH, :], 0.0)

            outr = out[:, :].rearrange("(lo li) j -> li lo j", li=P)

            # ---- pass A: K/V projections + Linformer reduction
            for lc in range(NLC):
                if lc == 0:
                    # chunk 0 split dc0-first so the first matmul gates on the
                    # smallest possible transfer; chunk 1 prefetch also on the
                    # GpSimd queue. Chunks >=2 are issued on the Sync queue
                    # inside the PREVIOUS chunk's body, after its E load, so
                    # ring-FIFO order guarantees E is never starved by x.
                    nc.gpsimd.dma_start(xs_all[:, 0, 0:1], xT[0, :, 0:1])
                    nc.gpsimd.dma_start(xs_all[:, 0, 1:DC], xT[0, :, 1:DC])
                    nc.gpsimd.dma_start(xs_all[:, 1, 0:4], xT[1, :, 0:4])
                    nc.gpsimd.dma_start(xs_all[:, 1, 4:DC], xT[1, :, 4:DC])
                x_sb = xs_all[:, lc]
                kv_tiles = []
                if lc == 0:
                    # startup: all K matmuls of an lt-PAIR first (interleaved
                    # across the two psK banks so no same-bank bubble), V after
                    # — the PE then only gates on the Wk load, and Wv's 1 MiB
                    # has ~4us of K-work to land under.
                    psKs, psVs, kvs = [], [], []
                    for lt in range(LT4):
                        psKs.append(psA.tile([P, LCH], F32, tag="big", name=f"psK{lt}"))
                        psVs.append(psA.tile([P, LCH], F32, tag="big", name=f"psV{lt}"))
                        kvs.append(kvpool.tile([P, 2, LCH], BF16, tag="kv", name=f"kv{lt}"))
                    for lt0 in (0, 2):
                        for dc in range(DC):
                            for lt in (lt0, lt0 + 1):
                                nc.tensor.matmul(
                                    psKs[lt][:], x_sb[:, dc, lt, :],
                                    wk_sb[:, dc, :],
                                    start=(dc == 0), stop=(dc == DC - 1),
                                )
                        for dc in range(DC):
                            for lt in (lt0, lt0 + 1):
                                nc.tensor.matmul(
                                    psVs[lt][:], x_sb[:, dc, lt, :],
                                    wv_sb[:, dc, :],
                                    start=(dc == 0), stop=(dc == DC - 1),
                                )
                        for lt in (lt0, lt0 + 1):
                            nc.any.tensor_tensor(kvs[lt][:, 0, :], psKs[lt][:], bkB_sb[:], add)
                            nc.any.tensor_tensor(kvs[lt][:, 1, :], psVs[lt][:], bvB_sb[:], add)
                    kv_tiles = kvs
                else:
                    for lt in range(LT4):
                        psK = psA.tile([P, LCH], F32, tag="big")
                        psV = psA.tile([P, LCH], F32, tag="big")
                        # K/V interleaved per dc: alternating the two PSUM banks
                        # hides the same-bank accumulation bubble.
                        for dc in range(DC):
                            nc.tensor.matmul(
                                psK[:], x_sb[:, dc, lt, :],
                                wk_sb[:, dc, :],
                                start=(dc == 0), stop=(dc == DC - 1),
                            )
                            nc.tensor.matmul(
                                psV[:], x_sb[:, dc, lt, :],
                                wv_sb[:, dc, :],
                                start=(dc == 0), stop=(dc == DC - 1),
                            )
                        kv_sb = kvpool.tile([P, 2, LCH], BF16, tag="kv")
                        nc.any.tensor_tensor(kv_sb[:, 0, :], psK[:], bkB_sb[:], add)
                        nc.any.tensor_tensor(kv_sb[:, 1, :], psV[:], bvB_sb[:], add)
                        kv_tiles.append(kv_sb)
                e_sb = epool.tile([P, H, LT4, KK], BF16, tag="e")
                nc.sync.dma_start(e_sb[:, 0 : H // 2], eT[lc, :, 0 : H // 2])
                nc.sync.dma_start(e_sb[:, H // 2 : H], eT[lc, :, H // 2 : H])
                if 1 <= lc < NLC - 1:
                    nc.sync.dma_start(xs_all[:, lc + 1, 0:4], xT[lc + 1, :, 0:4])
                    nc.sync.dma_start(xs_all[:, lc + 1, 4:DC], xT[lc + 1, :, 4:DC])
                for h in range(H):
                    par = h % 2
                    acc = kvpP[h // 2][:, :, par * DH : (par + 1) * DH]
                    psKV = psB.tile([P, 2, DH], F32, tag="big")
                    for lt in range(LT4):
                        nc.tensor.matmul(
                            psKV[:], e_sb[:, h, lt, :],
                            kv_tiles[lt][:, :, h * DH : (h + 1) * DH],
                            start=(lt == 0), stop=(lt == LT4 - 1),
                        )
                    if lc == 0:
                        nc.any.tensor_copy(acc, psKV[:])
                    else:
                        nc.any.tensor_tensor(acc, acc, psKV[:], add)
                if lc == 2:
                    # wq / bqT are first needed in pass B (~t+120us); issue on
                    # the Scalar queue (idle in pass A), deferred past the
                    # startup crunch so the 1 MiB transfer doesn't steal HBM
                    # bandwidth from the pass-A-critical loads.
                    nc.scalar.dma_start(wq_sb[:], wqP[:, :, :])
                    nc.scalar.dma_start(bqT_sb[:], bqT[:, :])

            # ---- between passes: Kp.T / Vp_aug staging
            for t in range(JT):
                # transpose both heads of the pair at once: [kk, dh2] -> [dh2, kk];
                # odd head's rows land at partitions 64..127 by construction
                psT = psB.tile([P, KK], F32, tag="big")
                nc.tensor.transpose(psT[:], kvpP[t][:, 0, :], ident[:])
                for par in range(2):
                    h = 2 * t + par
                    b0 = par * DH
                    nc.any.tensor_copy(kpT[h][b0 : b0 + DH, :], psT[b0 : b0 + DH, :])
                    nc.any.tensor_copy(
                        vpa[h][:, 0:DH], kvpP[t][:, 1, b0 : b0 + DH]
                    )
                    nc.vector.memset(vpa[h][:, DH : DH + 1], 1.0)

            # ---- pass B: Q projection fused with attention, per chunk
            DEPTH = 3  # psD/exp issued this many heads ahead of psX
            for lc in range(NLC):
                x_sb = xs_all[:, lc]
                qt = qtpool.tile([P, JT, LCH], BF16, tag="qt")
                # jt-pairs interleaved across the two psB banks (same-bank
                # accumulation bubble, as above). psQ draws from psB so psD
                # keeps all 4 psA banks: dot_h then only waits on exp_{h-4}.
                for jt0 in (0, 2):
                    psQs = [psB.tile([P, LCH], F32, tag="big", name=f"psQ{i}") for i in range(2)]
                    for dc in range(DC):
                        for i in range(2):
                            jt = jt0 + i
                            nc.tensor.matmul(
                                psQs[i][:], wq_sb[:, dc, jt * P : (jt + 1) * P],
                                x_sb[:, dc],
                                start=(dc == 0), stop=(dc == DC - 1),
                            )
                    for i in range(2):
                        jt = jt0 + i
                        # bias-add on the Scalar engine (ACT: out = f(in*1+bias))
                        # — GpSimd can't read PSUM and the DVE queue would make
                        # psQ bank recycling wait behind the previous chunk's
                        # reciprocal/mult ops.
                        nc.scalar.activation(
                            qt[:, jt, :], psQs[i][:],
                            mybir.ActivationFunctionType.Identity,
                            bias=bqT_sb[:, jt : jt + 1],
                        )
                ot = outpool.tile([P, LT4, J], F32, tag="ot")
                exs = [None] * H
                for hh in range(H + DEPTH):
                    if hh < H:
                        h = hh
                        psD = psA.tile([P, LCH], F32, tag="big")
                        nc.tensor.matmul(
                            psD[:], kpT[h][:],
                            qt[:, h // 2, :],
                            start=True, stop=True,
                        )
                        ex = exppool.tile([P, LCH], BF16, tag="ex")
                        nc.scalar.activation(
                            ex[:], psD[:], mybir.ActivationFunctionType.Exp
                        )
                        exs[h] = ex
                    if hh >= DEPTH:
                        h = hh - DEPTH
                        ex = exs[h]
                        psX = psXp.tile([P, LT4, DH + 1], F32, tag="x4")
                        for lt in range(LT4):
                            nc.tensor.matmul(
                                psX[:, lt, :], ex[:, lt * P : (lt + 1) * P],
                                vpa[h][:],
                                start=True, stop=True,
                            )
                        rc = recpool.tile([P, LT4, 1], F32, tag="rc")
                        nc.vector.reciprocal(rc[:], psX[:, :, DH : DH + 1])
                        nc.vector.tensor_tensor(
                            ot[:, :, h * DH : (h + 1) * DH],
                            psX[:, :, 0:DH],
                            rc[:].to_broadcast([P, LT4, DH]),
                            mult,
                        )
                        if h % 2 == 1:
                            j0 = (h - 1) * DH
                            nc.sync.dma_start(
                                outr[:, lc * LT4 : (lc + 1) * LT4, j0 : j0 + 2 * DH],
                                ot[:, :, j0 : j0 + 2 * DH],
                            )

    return nc


def _get_program():
    global _PROGRAM
    if _PROGRAM is None:
        _PROGRAM = _build_program()
    return _PROGRAM


def kernel(x, Wq, bq, Wk, bk, Wv, bv, E):
    global LAST_RESULTS
    x = np.asarray(x, dtype=np.float32)
    Wq = np.asarray(Wq, dtype=np.float32)
    bq = np.asarray(bq, dtype=np.float32)
    Wk = np.asarray(Wk, dtype=np.float32)
    bk = np.asarray(bk, dtype=np.float32)
    Wv = np.asarray(Wv, dtype=np.float32)
    bv = np.asarray(bv, dtype=np.float32)
    E = np.asarray(E, dtype=np.float32)

    BF = ml_dtypes.bfloat16
    scale = 1.0 / math.sqrt(DH)
    # [d, l] -> [lc, lt, pi, dc, ll] (d = dc*128 + pi, l = lc*512 + lt*128 + ll)
    # [d, l] -> [lc, pi, dc, lt, ll] (d = dc*128 + pi, l = lc*512 + lt*128 + ll)
    xTs = [
        np.ascontiguousarray(
            x[b].T.reshape(DC, P, NLC, LT4, P).transpose(2, 1, 0, 3, 4).astype(BF)
        )
        for b in range(B)
    ]
    in_maps = []
    for core in range(NCORES):
        b = core % B
        hg = core // B
        js = slice(hg * J, (hg + 1) * J)
        hs = slice(hg * H, (hg + 1) * H)
        # W.T [D, J] -> [pi, dc, j] so per-partition lines are contiguous
        wqPs = np.ascontiguousarray(
            (Wq[js, :] * scale).T.reshape(DC, P, J).transpose(1, 0, 2).astype(BF)
        )
        wkPs = np.ascontiguousarray(
            Wk[js, :].T.reshape(DC, P, J).transpose(1, 0, 2).astype(BF)
        )
        wvPs = np.ascontiguousarray(
            Wv[js, :].T.reshape(DC, P, J).transpose(1, 0, 2).astype(BF)
        )
        bqTs = np.ascontiguousarray((bq[js] * scale).reshape(JT, P).T)
        bkBs = np.ascontiguousarray(np.broadcast_to(bk[js], (P, J)).astype(BF))
        bvBs = np.ascontiguousarray(np.broadcast_to(bv[js], (P, J)).astype(BF))
        E_s = E[hs]  # [H, KK, L]
        eTs = np.ascontiguousarray(
            E_s.reshape(H, KK, NLC, LT4, P).transpose(2, 4, 0, 3, 1).astype(BF)
        )  # [NLC, P, H, LT4, KK]
        in_maps.append(
            {
                "xT": xTs[b],
                "wqP": wqPs,
                "wkP": wkPs,
                "wvP": wvPs,
                "bqT": bqTs,
                "bkB": bkBs,
                "bvB": bvBs,
                "eT": eTs,
            }
        )

    nc = _get_program()
    res = run_bass_kernel_spmd(nc, in_maps, list(range(NCORES)), trace=TRACE)
    LAST_RESULTS = res

    outp = np.empty((B, L, D), dtype=np.float32)
    for core in range(NCORES):
        b = core % B
        hg = core // B
        outp[b, :, hg * J : (hg + 1) * J] = res.results[core]["out"]
    return outp
